# revision 25
# baseline (speedup 1.0000x reference)
"""Causal single-head attention (B=4, T=4096, C=1024, H=64) on 8 TRN2 NeuronCores.

Sharding: core = 2*b + h handles batch b, t-half h (rows [h*2048, (h+1)*2048)).
Uniform SPMD program per core:
  - triangle: causal attention within the own t-half (s, t both in own half)
  - rect: S^T[s in [0,2048), t in [2048+off, 2048+off+1024)], off = (pid%2)*1024
    (lower-half keys attending into upper-half queries, t-split across the pair)

v2 design:
  - x loaded per t-tile with SWDGE cast f32->bf16 into SBUF, then one 3-D xbar
    SBUF->SBUF DMA transpose per tile: xT[p, cb, t] = x[t, 128*cb + p].
  - v projected in vT form (wide moving operand), then xbar-transposed into
    v_own[s, h] layout with an appended ones column (softmax denominator).
  - QK^T runs as row-packed pairs: two concurrent K=64 matmuls on row groups
    (0,0)/(64,0), needing q/k duplicated into both partition halves.
  - AV computed transposed: outT[h, t] += v'[s, h].T @ attT[s, t] with 512-wide
    bf16 moving operand; row 64 of the accumulator is the denominator.
  - Rect partials pair-AllGathered; merged into trind by a conditional
    accumulate-DMA (only on the upper-half core). Final divide via reciprocal
    of the den row + K=1 outer-product broadcast matmul. Output written as
    outT [64, 2048]; the host transposes.
Softmax uses no max-subtraction (logits are O(6)).
"""
import sys

sys.path.insert(0, "/opt/trn_rl_repo")

from contextlib import ExitStack

import numpy as np

import concourse.bass as bass
import concourse.mybir as mybir
import concourse.tile as tile
from concourse import bacc
from concourse.bass_utils import run_bass_kernel_spmd

B, T, C, H = 4, 4096, 1024, 64
P = 128
HALF = T // 2              # 2048 rows per core
NB_C = C // P              # 8 contraction tiles
NT = HALF // P             # 16 own t/s tiles
RW = 1024                  # rect t-width per core
SCALE = float(H) ** -0.5
NEG = -1e9
F32, F32R, BF16 = mybir.dt.float32, mybir.dt.float32r, mybir.dt.bfloat16
N_CORES = 8
PAIRS = [[2 * b, 2 * b + 1] for b in range(B)]

# triangle attT storage: s-tile i holds t-cols [256*(i//2), 2048)
TRI_BASE = [256 * (i // 2) for i in range(NT)]
TRI_W = [HALF - b for b in TRI_BASE]
TRI_OFF = np.concatenate([[0], np.cumsum(TRI_W)]).tolist()
TRI_TOTAL = TRI_OFF[-1]  # 18432
VROW = H + 16               # v_own/v_rect row stride: 160B, 32B-aligned

_CACHE = {}
BODY_REPEAT = 1            # for differential timing in test.py
PHASES = set(range(1, 10))  # ablation for phase timing
SCHEDULE = None            # list of phase-sets, one body emission each
DEBUG_DUMPS = False        # emit intermediate tensors as extra outputs


def build():
    nc = bacc.Bacc("TRN2", target_bir_lowering=False, debug=False,
                   num_devices=N_CORES)
    x = nc.dram_tensor("x", [HALF, C], F32, kind="ExternalInput").ap()
    wq = nc.dram_tensor("wq", [C, H], F32, kind="ExternalInput").ap()
    wk = nc.dram_tensor("wk", [C, H], F32, kind="ExternalInput").ap()
    wv = nc.dram_tensor("wv", [C, H], F32, kind="ExternalInput").ap()
    trimask = nc.dram_tensor("trimask", [P, P], F32, kind="ExternalInput").ap()
    ident = nc.dram_tensor("ident", [H, H], F32, kind="ExternalInput").ap()
    outT = nc.dram_tensor("outT", [H, HALF], F32, kind="ExternalOutput").ap()

    EXP = mybir.ActivationFunctionType.Exp

    with tile.TileContext(nc) as tc, ExitStack() as ctx:
        big = ctx.enter_context(tc.tile_pool(name="big", bufs=1))
        stage = ctx.enter_context(tc.tile_pool(name="stage", bufs=2))
        ps = ctx.enter_context(tc.tile_pool(name="ps", bufs=2, space="PSUM"))
        dram = ctx.enter_context(tc.tile_pool(name="dram", bufs=1, space="DRAM"))

        # ---- constants ----
        tri_sb = big.tile([P, P], F32, tag="tri")
        nc.sync.dma_start(tri_sb[:], trimask[:])
        wqk_sb = big.tile([P, NB_C, 2 * H], BF16, tag="wqk")
        nc.gpsimd.dma_start(wqk_sb[:, :, 0:H], wq.rearrange("(cb p) h -> p cb h", p=P))
        nc.gpsimd.dma_start(wqk_sb[:, :, H:2 * H], wk.rearrange("(cb p) h -> p cb h", p=P))
        wv_sb = big.tile([P, NB_C, H], BF16, tag="wv")
        nc.gpsimd.dma_start(wv_sb[:], wv.rearrange("(cb p) h -> p cb h", p=P))
        # ones row on partition H (=64): lhsT for the den-broadcast matmul must
        # share the contraction partition with the den row of trind
        ones_sb = big.tile([H + 1, H], F32, tag="ones")
        nc.vector.memset(ones_sb[H:H + 1, :], 1.0)
        ones_r = big.tile([H + 1, H], F32R, tag="ones_r")
        nc.vector.tensor_copy(ones_r[H:H + 1, :], ones_sb[H:H + 1, :])
        ident_sb = big.tile([H, H], BF16, tag="ident")
        nc.gpsimd.dma_start(ident_sb[:], ident[:])

        pid = nc.partition_id(engines=[mybir.EngineType.Pool])
        qoff = (pid % 2) * RW
        is_h1 = pid % 2

        schedule = SCHEDULE if SCHEDULE is not None else [PHASES] * BODY_REPEAT
        for _rep in range(len(schedule)):
            cur = schedule[_rep]
            if 1 in cur:
                # ---- x load (cast bf16, 512-row quarters) + xbar per t-tile
                # (transposes alternate across both HWDGE rings) ----
                xT = big.tile([P, NB_C, HALF], BF16, tag="xT")
                for q in range(4):
                    xs = stage.tile([P, 4, C], BF16, tag="xs")
                    nc.gpsimd.dma_start(
                        xs[:], x[q * 512:(q + 1) * 512, :].rearrange(
                            "(a p) c -> p a c", p=P))
                    for a in range(4):
                        tt = 4 * q + a
                        nc.sync.dma_start(xT[:, :, tt * P:(tt + 1) * P],
                                          xs[:, a, :], transpose=True)

            if 2 in cur:
                # ---- projections: q|k packed wide; v direct in [t, h] ----
                qdup = big.tile([P, HALF], F32R, tag="qdup")
                kdup = big.tile([P, HALF], F32R, tag="kdup")
                for tg in range(4):
                    sl = slice(tg * 512, (tg + 1) * 512)
                    pqk = ps.tile([P, 512], F32, tag="a")
                    for cb in range(NB_C):
                        nc.tensor.matmul(pqk[:], wqk_sb[:, cb, :],
                                         xT[:, cb, sl],
                                         start=(cb == 0), stop=(cb == NB_C - 1))
                    nc.vector.tensor_copy(qdup[0:H, sl], pqk[0:H, :])
                    nc.vector.tensor_copy(kdup[H:P, sl], pqk[H:P, :])
                vT_sb = big.tile([H, HALF], BF16, tag="vT")
                for tg in range(4):
                    sl = slice(tg * 512, (tg + 1) * 512)
                    pv = ps.tile([H, 512], F32, tag="v")
                    for cb in range(NB_C):
                        nc.tensor.matmul(pv[:], wv_sb[:, cb, :],
                                         xT[:, cb, sl],
                                         start=(cb == 0), stop=(cb == NB_C - 1))
                    nc.vector.tensor_copy(vT_sb[:, sl], pv[:])
                v_own = big.tile([P, NT, VROW], BF16, tag="vown")
                nc.vector.memset(v_own[:, :, H:H + 1], 1.0)
                for tt in range(NT):
                    ptv = ps.tile([P, H], BF16, tag="vt")
                    nc.tensor.transpose(ptv[:], vT_sb[:, tt * P:(tt + 1) * P],
                                        ident_sb[:])
                    nc.vector.tensor_copy(v_own[:, tt, 0:H], ptv[:])
                # q|k and v out to DRAM (feeds the AllGather and the dups)
                qkb = dram.tile([2 * H, HALF], F32R)
                vb2 = dram.tile([HALF, H], BF16)
                nc.gpsimd.dma_start(qkb[0:H, :], qdup[0:H, :])
                nc.gpsimd.dma_start(qkb[H:2 * H, :], kdup[H:P, :])
                nc.gpsimd.dma_start(vb2.rearrange("(st p) h -> p st h", p=P),
                                    v_own[:, :, 0:H])
                # cross-partition dups via DRAM
                nc.sync.dma_start(qdup[H:P, :], qkb[0:H, :])
                nc.sync.dma_start(kdup[0:H, :], qkb[H:2 * H, :])

            if 3 in cur:
                # ---- pair collectives: gather q|k and v ----
                gqk = dram.tile([4 * H, HALF], F32R)
                gv2 = dram.tile([T, H], BF16)
                for src, dst in ((qkb, gqk), (vb2, gv2)):
                    nc.gpsimd.collective_compute(
                        "AllGather", mybir.AluOpType.bypass,
                        replica_groups=PAIRS,
                        ins=[src.opt()], outs=[dst.opt()])

            if 4 in cur:
                # ---- triangle QK^T as row-packed pairs + exp ----
                attT_tri = big.tile([P, TRI_TOTAL], BF16, tag="att_tri")
                for j in range(8):
                    base = 256 * j
                    i0, i1 = 2 * j, 2 * j + 1
                    for c0 in range(base, HALF, 512):
                        w = min(512, HALF - c0)
                        pa = ps.tile([P, 512], F32, tag="a")
                        pb = ps.tile([P, 512], F32, tag="b")
                        nc.tensor.matmul(pa[:, 0:w],
                                         kdup[0:H, i0 * P:(i0 + 1) * P],
                                         qdup[0:H, c0:c0 + w],
                                         start=True, stop=True)
                        nc.tensor.matmul(pb[:, 0:w],
                                         kdup[H:P, i1 * P:(i1 + 1) * P],
                                         qdup[H:P, c0:c0 + w],
                                         start=True, stop=True)
                        if c0 == base:
                            nc.vector.tensor_add(pa[:, 0:P], pa[:, 0:P], tri_sb[:])
                            nc.vector.tensor_add(pb[:, P:2 * P], pb[:, P:2 * P],
                                                 tri_sb[:])
                        d0 = c0 - base
                        nc.scalar.activation(
                            attT_tri[:, TRI_OFF[i0] + d0:TRI_OFF[i0] + d0 + w],
                            pa[:, 0:w], EXP, scale=SCALE)
                        nc.scalar.activation(
                            attT_tri[:, TRI_OFF[i1] + d0:TRI_OFF[i1] + d0 + w],
                            pb[:, 0:w], EXP, scale=SCALE)

            if 5 in cur:
                # ---- rect operands (from gathered) + rect QK^T pairs ----
                kdup_r = big.tile([P, HALF], F32R, tag="kdup_r")
                nc.sync.dma_start(kdup_r[0:H, :], gqk[H:2 * H, :])
                nc.sync.dma_start(kdup_r[H:P, :], gqk[H:2 * H, :])
                qdup_r = big.tile([P, RW], F32R, tag="qdup_r")
                nc.gpsimd.dma_start(qdup_r[0:H, :],
                                    gqk[2 * H:3 * H, bass.ds(qoff, RW)])
                nc.gpsimd.dma_start(qdup_r[H:P, :],
                                    gqk[2 * H:3 * H, bass.ds(qoff, RW)])
                v_rect = big.tile([P, NT, VROW], BF16, tag="vrect")
                nc.vector.memset(v_rect[:, :, H:H + 1], 1.0)
                nc.sync.dma_start(
                    v_rect[:, :, 0:H],
                    gv2[0:HALF, :].rearrange("(st p) h -> p st h", p=P))

                attT_rect = big.tile([P, NT * RW], BF16, tag="att_rect")
                for jp in range(8):
                    i0, i1 = 2 * jp, 2 * jp + 1
                    for g in range(2):
                        sl = slice(g * 512, (g + 1) * 512)
                        pa = ps.tile([P, 512], F32, tag="a")
                        pb = ps.tile([P, 512], F32, tag="b")
                        nc.tensor.matmul(pa[:], kdup_r[0:H, i0 * P:(i0 + 1) * P],
                                         qdup_r[0:H, sl], start=True, stop=True)
                        nc.tensor.matmul(pb[:], kdup_r[H:P, i1 * P:(i1 + 1) * P],
                                         qdup_r[H:P, sl], start=True, stop=True)
                        nc.scalar.activation(
                            attT_rect[:, i0 * RW + g * 512:i0 * RW + g * 512 + 512],
                            pa[:], EXP, scale=SCALE)
                        nc.scalar.activation(
                            attT_rect[:, i1 * RW + g * 512:i1 * RW + g * 512 + 512],
                            pb[:], EXP, scale=SCALE)

            if 7 in cur:
                # ---- rect AV (transposed, num|den via ones row) ----
                rectnd = big.tile([H + 1, RW], F32, tag="rectnd")
                for g in range(2):
                    pav = ps.tile([H + 1, 512], F32, tag="a")
                    for st in range(NT):
                        nc.tensor.matmul(
                            pav[:], v_rect[:, st, 0:H + 1],
                            attT_rect[:, st * RW + g * 512:st * RW + g * 512 + 512],
                            start=(st == 0), stop=(st == NT - 1))
                    nc.vector.tensor_copy(rectnd[:, g * 512:(g + 1) * 512], pav[:])

            if 8 in cur:
                # ---- partial exchange (flies during the triangle AV) ----
                ndb = dram.tile([H + 1, RW], F32)
                nc.gpsimd.dma_start(ndb[:], rectnd[:])
                gnd = dram.tile([2 * (H + 1), RW], F32)
                nc.gpsimd.collective_compute(
                    "AllGather", mybir.AluOpType.bypass, replica_groups=PAIRS,
                    ins=[ndb.opt()], outs=[gnd.opt()])

            if 6 in cur:
                # ---- triangle AV (transposed accumulation) ----
                trind = big.tile([H + 1, HALF], F32, tag="trind")
                for g in range(4):
                    pav = ps.tile([H + 1, 512], F32, tag="b")
                    last = 4 * g + 3
                    for st in range(last + 1):
                        cs = max(512 * g, P * st)
                        w = 512 * g + 512 - cs
                        col = TRI_OFF[st] + cs - TRI_BASE[st]
                        nc.tensor.matmul(pav[:, cs - 512 * g:512],
                                         v_own[:, st, 0:H + 1],
                                         attT_tri[:, col:col + w],
                                         start=(st == 0), stop=(st == last))
                    nc.vector.tensor_copy(trind[:, g * 512:(g + 1) * 512], pav[:])

            if 9 in cur:
                # ---- merge rect partials (upper core only) + divide ----
                nc.gpsimd.dma_start(trind[:, 0:RW], gnd[0:H + 1, :],
                                    accum_op=mybir.AluOpType.add, cond=is_h1)
                nc.gpsimd.dma_start(trind[:, RW:HALF], gnd[H + 1:2 * (H + 1), :],
                                    accum_op=mybir.AluOpType.add, cond=is_h1)
                recip_r = big.tile([H + 1, HALF], F32R, tag="recip_r")
                with nc.allow_low_precision(reason="f32r has full fp32 range"):
                    nc.vector.reciprocal(recip_r[H:H + 1, :], trind[H:H + 1, :])
                out_sb = big.tile([H, HALF], F32, tag="out_sb")
                for g in range(4):
                    sl = slice(g * 512, (g + 1) * 512)
                    pbc = ps.tile([H, 512], F32, tag="v")
                    nc.tensor.matmul(pbc[:], ones_r[H:H + 1, :],
                                     recip_r[H:H + 1, sl],
                                     start=True, stop=True)
                    nc.vector.tensor_mul(out_sb[:, sl], trind[0:H, sl], pbc[:])
                nc.sync.dma_start(outT[:], out_sb[:])

            if DEBUG_DUMPS:
                def dump(name, src_fn, shape, dt):
                    try:
                        src_ap = src_fn()
                    except NameError:
                        return
                    o = nc.dram_tensor(name, shape, dt,
                                       kind="ExternalOutput").ap()
                    nc.sync.dma_start(o[:], src_ap)
                dump("d_qdup", lambda: qdup[:], [P, HALF], F32R)
                dump("d_kdup", lambda: kdup[:], [P, HALF], F32R)
                dump("d_vown", lambda: v_own[:].rearrange("p a b -> p (a b)"),
                     [P, NT * VROW], BF16)
                dump("d_atttri0", lambda: attT_tri[:, 0:2048], [P, 2048], BF16)
                dump("d_attrect0", lambda: attT_rect[:, 0:RW], [P, RW], BF16)
                dump("d_rectnd", lambda: rectnd[:], [H + 1, RW], F32)
                dump("d_trind", lambda: trind[:], [H + 1, HALF], F32)

    nc.compile()
    return nc


def make_in_maps(x, Wq, Wk, Wv):
    x = np.asarray(x, dtype=np.float32)
    Wq = np.asarray(Wq, dtype=np.float32)
    Wk = np.asarray(Wk, dtype=np.float32)
    Wv = np.asarray(Wv, dtype=np.float32)
    # S^T layout: partition=s, free=t; allowed s<=t -> tri[s,t]=0 iff s<=t
    tri = np.where(np.arange(P)[:, None] <= np.arange(P)[None, :], 0.0,
                   NEG).astype(np.float32)
    in_maps = []
    for c in range(N_CORES):
        b, h = c // 2, c % 2
        in_maps.append({
            "x": np.ascontiguousarray(x[b, h * HALF:(h + 1) * HALF, :]),
            "wq": Wq, "wk": Wk, "wv": Wv,
            "trimask": tri,
            "ident": np.eye(H, dtype=np.float32),
        })
    return in_maps


def kernel(x, Wq, Wk, Wv):
    if "nc" not in _CACHE:
        _CACHE["nc"] = build()
    nc = _CACHE["nc"]
    in_maps = make_in_maps(x, Wq, Wk, Wv)
    res = None
    for attempt in range(4):
        try:
            res = run_bass_kernel_spmd(nc, in_maps, list(range(N_CORES)))
            break
        except Exception:
            if attempt == 3:
                raise
            import time as _time
            _time.sleep(5)
    out = np.empty((B, T, H), np.float32)
    for c in range(N_CORES):
        b, h = c // 2, c % 2
        out[b, h * HALF:(h + 1) * HALF, :] = res.results[c]["outT"].T
    return out
